# revision 8
# baseline (speedup 1.0000x reference)
"""Trainium2 Bass kernel for nn_CrossAttention_65051574665735.

Cross-attention block (MQA, shared K/V head) + parallel SwiGLU FF.
Data-parallel over B*N rows across 8 NeuronCores: core c handles batch c//4,
rows (c%4)*512. Context + weights replicated (weights pre-cast to bf16 with the
layernorm scale g and the 1/sqrt(dh) attention scale folded in on the host).
No cross-core collectives; the host concatenates the 8 output slices.

Schedule: prologue (x-LN/transpose, Q proj, ctx-LN/KV) -> attention (ACT
exp-bound; score matmul pairs run row-tiled concurrently; FF1 matmuls
interleaved as PE filler) -> tail (FF2+Wo fused accumulation per row-slice).
"""

import sys

if "/opt/trn_rl_repo" not in sys.path:
    sys.path.insert(0, "/opt/trn_rl_repo")

import numpy as np
import ml_dtypes

import concourse.bass as bass
import concourse.tile as tile
from concourse import mybir, bacc
from concourse.masks import make_identity

F32 = mybir.dt.float32
BF16 = mybir.dt.bfloat16

B, N, J = 2, 2048, 2048
DIM, HEADS, DH = 1024, 16, 64
INNER = HEADS * DH
FF = 4 * DIM
EPS = 1e-5
N_CORES = 8
R = B * N // N_CORES  # 512 rows per core
KT = DIM // 128  # 8 contraction tiles over dim
RT = R // 128  # 4 row tiles
CT = J // 128  # 16 context row tiles
FT = FF // 128  # 32 ff tiles
AF = mybir.ActivationFunctionType


def _ln_tile(nc, pools, src_dram, t, dst_tiles, dst_col0, bias_tile):
    """LN one 128-row tile of src_dram; write transposed bf16 into dst_tiles.

    dst_tiles[k][:, dst_col0:dst_col0+128] gets chunk k of the transposed
    normalized rows. Stats on DVE, normalize on ACT, transposes on PE,
    psum->sbuf copies on DVE.
    """
    ln_pool, stats_pool, psum_tr, ident, eps_tile = pools
    x_t = ln_pool.tile([128, DIM], F32, tag="ln_x")
    nc.gpsimd.dma_start(x_t[:], src_dram[t * 128 : (t + 1) * 128, :])
    stats = stats_pool.tile([128, 2, nc.vector.BN_STATS_DIM], F32, tag="st")
    nc.vector.bn_stats(stats[:, 0, :], x_t[:, 0:512])
    nc.vector.bn_stats(stats[:, 1, :], x_t[:, 512:1024])
    mv = stats_pool.tile([128, nc.vector.BN_AGGR_DIM], F32, tag="mv")
    nc.vector.bn_aggr(mv[:], stats[:])
    rstd = stats_pool.tile([128, 1], F32, tag="rs")
    nc.scalar.activation(rstd[:], mv[:, 1:2], AF.Sqrt, bias=eps_tile[:])
    nc.vector.reciprocal(rstd[:], rstd[:])
    nmr = stats_pool.tile([128, 1], F32, tag="nmr")
    nc.vector.tensor_scalar(
        out=nmr[:],
        in0=mv[:, 0:1],
        scalar1=rstd[:, 0:1],
        scalar2=-1.0,
        op0=mybir.AluOpType.mult,
        op1=mybir.AluOpType.mult,
    )
    xn_t = ln_pool.tile([128, DIM], BF16, tag="ln_xn")
    nc.scalar.activation(
        xn_t[:], x_t[:], AF.Identity, bias=nmr[:, 0:1], scale=rstd[:, 0:1]
    )
    if bias_tile is not None:
        nc.vector.tensor_add(xn_t[:], xn_t[:], bias_tile[:])
    for k in range(KT):
        ps = psum_tr.tile([128, 128], BF16, tag="tr")
        nc.tensor.transpose(ps[:], xn_t[:, k * 128 : (k + 1) * 128], ident[:])
        nc.vector.tensor_copy(
            dst_tiles[k][:, dst_col0 : dst_col0 + 128], ps[:]
        )


def build_kernel(x_bias_nonzero: bool, c_bias_nonzero: bool):
    nc = bacc.Bacc(
        "TRN2", target_bir_lowering=False, debug=False, num_devices=N_CORES
    )
    d_x = nc.dram_tensor("x", [R, DIM], F32, kind="ExternalInput").ap()
    d_ctx = nc.dram_tensor("ctx", [J, DIM], F32, kind="ExternalInput").ap()
    d_wq = nc.dram_tensor("wq", [DIM, INNER], BF16, kind="ExternalInput").ap()
    d_wkv = nc.dram_tensor("wkv", [DIM, 2 * DH], BF16, kind="ExternalInput").ap()
    d_wo = nc.dram_tensor("wo", [INNER, DIM], BF16, kind="ExternalInput").ap()
    d_wff1 = nc.dram_tensor("wff1", [DIM, 2 * FF], BF16, kind="ExternalInput").ap()
    d_wff2 = nc.dram_tensor("wff2", [FF, DIM], BF16, kind="ExternalInput").ap()
    d_xb = (
        nc.dram_tensor("xb", [1, DIM], F32, kind="ExternalInput").ap()
        if x_bias_nonzero
        else None
    )
    d_cb = (
        nc.dram_tensor("cb", [1, DIM], F32, kind="ExternalInput").ap()
        if c_bias_nonzero
        else None
    )
    d_out = nc.dram_tensor("out", [R, DIM], F32, kind="ExternalOutput").ap()

    with tile.TileContext(nc) as tc:
        with (
            tc.tile_pool(name="consts", bufs=1) as consts,
            tc.tile_pool(name="persist", bufs=1) as persist,
            tc.tile_pool(name="ln", bufs=3) as ln_pool,
            tc.tile_pool(name="stats", bufs=4) as stats_pool,
        ):
            ident = consts.tile([128, 128], BF16)
            make_identity(nc, ident)
            eps_tile = consts.tile([128, 1], F32, tag="eps")
            nc.vector.memset(eps_tile[:], EPS)

            xb_tile = cb_tile = None
            if d_xb is not None:
                xb_tile = consts.tile([128, DIM], F32, tag="xb")
                nc.gpsimd.dma_start(
                    xb_tile[:],
                    bass.AP(
                        tensor=d_xb.tensor, offset=d_xb.offset,
                        ap=[[0, 128]] + d_xb.ap[1:],
                    ),
                )
            if d_cb is not None:
                cb_tile = consts.tile([128, DIM], F32, tag="cb")
                nc.gpsimd.dma_start(
                    cb_tile[:],
                    bass.AP(
                        tensor=d_cb.tensor, offset=d_cb.offset,
                        ap=[[0, 128]] + d_cb.ap[1:],
                    ),
                )

            xnT = [
                persist.tile([128, R], BF16, tag=f"xnT{k}", name=f"xnT{k}")
                for k in range(KT)
            ]
            kT = persist.tile([128, J], BF16, tag="kT")
            vo = [
                persist.tile([128, DH + 1], BF16, tag=f"vo{j}", name=f"vo{j}")
                for j in range(CT)
            ]
            aoT = [
                persist.tile([128, R], BF16, tag=f"aoT{k}", name=f"aoT{k}")
                for k in range(KT)
            ]
            hT = [
                persist.tile([128, R], BF16, tag=f"hT{f}", name=f"hT{f}")
                for f in range(FT)
            ]
            qT = [
                persist.tile([128, R], BF16, tag=f"qT{h}", name=f"qT{h}")
                for h in range(HEADS // 2)
            ]

            # ---- FF1 pools span prologue + attention (PE gap filler) ----
            with (
                tc.tile_pool(name="wff1", bufs=24) as wff1_pool,
                tc.tile_pool(name="sg", bufs=3) as sg_pool,
                tc.tile_pool(name="psF", bufs=1, space="PSUM") as psum_f,
            ):

                def ff1_block(w1a, w1g, fl, fi):
                    a_ps = psum_f.tile([128, R], F32, tag="ffa", name="a_ps")
                    g_ps = psum_f.tile([128, R], F32, tag="ffg", name="g_ps")
                    for k in range(KT):
                        nc.tensor.matmul(
                            a_ps[:],
                            w1a[k][:, fl * 128 : (fl + 1) * 128],
                            xnT[k][:],
                            start=(k == 0),
                            stop=(k == KT - 1),
                        )
                    for k in range(KT):
                        nc.tensor.matmul(
                            g_ps[:],
                            w1g[k][:, fl * 128 : (fl + 1) * 128],
                            xnT[k][:],
                            start=(k == 0),
                            stop=(k == KT - 1),
                        )
                    sg = sg_pool.tile([128, R], F32, tag="sg", name="sg")
                    nc.scalar.activation(sg[:], g_ps[:], AF.Silu)
                    nc.vector.tensor_mul(hT[fi][:], a_ps[:], sg[:])

                def load_w1_group(g):
                    pair = ([], [])
                    for k in range(KT):
                        ta = wff1_pool.tile(
                            [128, 1024], BF16, tag="w1", name=f"w1a{g}_{k}"
                        )
                        nc.sync.dma_start(
                            ta[:],
                            d_wff1[
                                k * 128 : (k + 1) * 128,
                                g * 1024 : (g + 1) * 1024,
                            ],
                        )
                        pair[0].append(ta)
                        tg = wff1_pool.tile(
                            [128, 1024], BF16, tag="w1", name=f"w1g{g}_{k}"
                        )
                        nc.sync.dma_start(
                            tg[:],
                            d_wff1[
                                k * 128 : (k + 1) * 128,
                                FF + g * 1024 : FF + (g + 1) * 1024,
                            ],
                        )
                        pair[1].append(tg)
                    return pair

                w1 = {}
                with tc.tile_pool(name="psA", bufs=2, space="PSUM") as psum_tr:
                    pools = (ln_pool, stats_pool, psum_tr, ident, eps_tile)
                    # ---- Prologue A: x LN+transpose, all Q projections ----
                    with (
                        tc.tile_pool(name="wq", bufs=1) as wq_pool,
                        tc.tile_pool(name="psQ", bufs=2, space="PSUM") as psum_q,
                    ):
                        wq_sb = [
                            wq_pool.tile(
                                [128, INNER], BF16, tag=f"wq{k}", name=f"wq{k}"
                            )
                            for k in range(KT)
                        ]
                        for k in range(KT):
                            nc.sync.dma_start(
                                wq_sb[k][:], d_wq[k * 128 : (k + 1) * 128, :]
                            )
                        for t in range(RT):
                            _ln_tile(nc, pools, d_x, t, xnT, t * 128, xb_tile)
                        for hp in range(HEADS // 2):
                            q_ps = psum_q.tile([128, R], F32, tag="q", name="q_ps")
                            for k in range(KT):
                                nc.tensor.matmul(
                                    q_ps[:],
                                    wq_sb[k][:, hp * 128 : (hp + 1) * 128],
                                    xnT[k][:],
                                    start=(k == 0),
                                    stop=(k == KT - 1),
                                )
                            nc.vector.tensor_copy(qT[hp][:], q_ps[:])

                    # ---- Prologue B: ctx LN+transpose, KV, first FF1 ----
                    with (
                        tc.tile_pool(name="cnT", bufs=1) as cnT_pool,
                        tc.tile_pool(name="wkv", bufs=1) as wkv_pool,
                        tc.tile_pool(name="psKV", bufs=2, space="PSUM") as psum_kv,
                        tc.tile_pool(name="vstage", bufs=2) as vstage,
                    ):
                        wkv_sb = [
                            wkv_pool.tile(
                                [128, 2 * DH], BF16, tag=f"wkv{k}", name=f"wkv{k}"
                            )
                            for k in range(KT)
                        ]
                        for k in range(KT):
                            nc.sync.dma_start(
                                wkv_sb[k][:], d_wkv[k * 128 : (k + 1) * 128, :]
                            )
                        # FF1 weight groups 0/1 stream during ctx LN
                        w1[0] = load_w1_group(0)
                        w1[1] = load_w1_group(1)
                        cnT = [
                            cnT_pool.tile(
                                [128, J], BF16, tag=f"cnT{k}", name=f"cnT{k}"
                            )
                            for k in range(KT)
                        ]
                        for t in range(CT):
                            _ln_tile(nc, pools, d_ctx, t, cnT, t * 128, cb_tile)
                        for jc in range(CT):
                            nc.vector.memset(vo[jc][:, DH : DH + 1], 1.0)
                        for c in range(J // 512):
                            kv_ps = psum_kv.tile(
                                [128, 512], F32, tag="kv", name="kv_ps"
                            )
                            for k in range(KT):
                                nc.tensor.matmul(
                                    kv_ps[:],
                                    wkv_sb[k][:],
                                    cnT[k][:, c * 512 : (c + 1) * 512],
                                    start=(k == 0),
                                    stop=(k == KT - 1),
                                )
                            # rows 0:64 = kT chunk; duplicate into 64:128 for
                            # the row-tiled h2=1 score matmuls
                            nc.scalar.copy(
                                kT[0:DH, c * 512 : (c + 1) * 512], kv_ps[0:DH, :]
                            )
                            nc.gpsimd.dma_start(
                                kT[DH:128, c * 512 : (c + 1) * 512],
                                kT[0:DH, c * 512 : (c + 1) * 512],
                            )
                            vT_sb = vstage.tile([128, 512], BF16, tag="vT")
                            nc.vector.tensor_copy(
                                vT_sb[DH:128, :], kv_ps[DH:128, :]
                            )
                            for j4 in range(4):
                                jc = c * 4 + j4
                                vps = psum_tr.tile([128, DH], BF16, tag="tr")
                                nc.tensor.transpose(
                                    vps[:],
                                    vT_sb[DH:128, j4 * 128 : (j4 + 1) * 128],
                                    ident[DH:128, DH:128],
                                )
                                nc.vector.tensor_copy(vo[jc][:, 0:DH], vps[:])
                        for fl in range(4):
                            ff1_block(w1[0][0], w1[0][1], fl, fl)

                # ---- Attention (exp-bound) with FF1 blocks interleaved ----
                with (
                    tc.tile_pool(name="attnT", bufs=4) as attn_pool,
                    tc.tile_pool(name="smx", bufs=4) as smx_pool,
                    tc.tile_pool(name="psS", bufs=2, space="PSUM") as psum_s,
                    tc.tile_pool(name="psAV", bufs=2, space="PSUM") as psum_av,
                ):

                    def attn_block(hp):
                        a0s, a1s = [], []
                        for jp in range(CT // 2):
                            s0 = psum_s.tile(
                                [128, 2 * 512], F32, tag="sim", name="s0"
                            )
                            s1 = psum_s.tile(
                                [128, 2 * 512], F32, tag="sim", name="s1"
                            )
                            for half in range(2):
                                jc = 2 * jp + half
                                nc.tensor.matmul(
                                    s0[:, half * 512 : (half + 1) * 512],
                                    kT[0:DH, jc * 128 : (jc + 1) * 128],
                                    qT[hp][0:DH, :],
                                    start=True,
                                    stop=True,
                                )
                                nc.tensor.matmul(
                                    s1[:, half * 512 : (half + 1) * 512],
                                    kT[DH:128, jc * 128 : (jc + 1) * 128],
                                    qT[hp][DH:128, :],
                                    start=True,
                                    stop=True,
                                )
                            a0 = attn_pool.tile(
                                [128, 2 * 512], BF16, tag="attnT", name="a0"
                            )
                            nc.scalar.activation(a0[:], s0[:], AF.Exp)
                            a1 = attn_pool.tile(
                                [128, 2 * 512], BF16, tag="attnT", name="a1"
                            )
                            nc.scalar.activation(a1[:], s1[:], AF.Exp)
                            a0s.append(a0)
                            a1s.append(a1)
                        av0 = psum_av.tile([DH + 1, R], F32, tag="av", name="av0")
                        av1 = psum_av.tile([DH + 1, R], F32, tag="av", name="av1")
                        for jc in range(CT):
                            nc.tensor.matmul(
                                av0[:],
                                vo[jc][:],
                                a0s[jc // 2][
                                    :, (jc % 2) * 512 : (jc % 2 + 1) * 512
                                ],
                                start=(jc == 0),
                                stop=(jc == CT - 1),
                            )
                        for jc in range(CT):
                            nc.tensor.matmul(
                                av1[:],
                                vo[jc][:],
                                a1s[jc // 2][
                                    :, (jc % 2) * 512 : (jc % 2 + 1) * 512
                                ],
                                start=(jc == 0),
                                stop=(jc == CT - 1),
                            )
                        for h2, av in ((0, av0), (1, av1)):
                            srow = smx_pool.tile([1, R], F32, tag="srow")
                            nc.vector.tensor_copy(srow[:], av[DH : DH + 1, :])
                            rbc = smx_pool.tile([DH, R], F32, tag="rbc")
                            nc.gpsimd.partition_broadcast(rbc[:], srow[:])
                            nc.vector.reciprocal(rbc[:], rbc[:])
                            if h2 == 0:
                                nc.vector.tensor_mul(
                                    aoT[hp][0:DH, :], av[0:DH, :], rbc[:]
                                )
                            else:
                                tmp = smx_pool.tile([DH, R], BF16, tag="aotmp")
                                nc.vector.tensor_mul(tmp[:], av[0:DH, :], rbc[:])
                                nc.gpsimd.dma_start(aoT[hp][DH:128, :], tmp[:])

                    # remaining FF1 work: g0 fl 4..7, g1, g2, g3 = 28 blocks
                    ff_sched = [(0, fl) for fl in range(4, 8)]
                    ff_sched += [(g, fl) for g in (1, 2, 3) for fl in range(8)]
                    fi_iter = iter(ff_sched)
                    for hp in range(HEADS // 2):
                        if hp == 2:
                            w1[2] = load_w1_group(2)
                        if hp == 4:
                            w1[3] = load_w1_group(3)
                        attn_block(hp)
                        # ~3.5 FF1 blocks per head-pair keeps PE fed while
                        # ACT works through the exps
                        nblk = 4 if hp % 2 == 0 else 3
                        for _ in range(nblk):
                            gfl = next(fi_iter, None)
                            if gfl is None:
                                continue
                            g, fl = gfl
                            ff1_block(w1[g][0], w1[g][1], fl, g * 8 + fl)
                    for gfl in fi_iter:
                        g, fl = gfl
                        ff1_block(w1[g][0], w1[g][1], fl, g * 8 + fl)

            # ---- Tail: FF2 (start) + Wo (stop) per row-slice ----
            with (
                tc.tile_pool(name="wff2", bufs=1) as wff2_pool,
                tc.tile_pool(name="wo", bufs=1) as wo_pool,
                tc.tile_pool(name="psO", bufs=4, space="PSUM") as psum_o,
                tc.tile_pool(name="ostage", bufs=2) as ostage,
            ):
                w2 = []
                for fi in range(FT):
                    t2 = wff2_pool.tile(
                        [128, DIM], BF16, tag=f"w2_{fi}", name=f"w2_{fi}"
                    )
                    nc.sync.dma_start(
                        t2[:], d_wff2[fi * 128 : (fi + 1) * 128, :]
                    )
                    w2.append(t2)
                wo_sb = [
                    wo_pool.tile([128, DIM], BF16, tag=f"wo{k}", name=f"wo{k}")
                    for k in range(KT)
                ]
                for k in range(KT):
                    nc.sync.dma_start(
                        wo_sb[k][:], d_wo[k * 128 : (k + 1) * 128, :]
                    )
                for rs in range(RT):
                    o_ps = [
                        psum_o.tile([128, 512], F32, tag="o", name=f"o{rs}{ch}")
                        for ch in range(2)
                    ]
                    for fi in range(FT):
                        for ch in range(2):
                            nc.tensor.matmul(
                                o_ps[ch][:],
                                hT[fi][:, rs * 128 : (rs + 1) * 128],
                                w2[fi][:, ch * 512 : (ch + 1) * 512],
                                start=(fi == 0),
                                stop=False,
                            )
                    for k in range(KT):
                        for ch in range(2):
                            nc.tensor.matmul(
                                o_ps[ch][:],
                                aoT[k][:, rs * 128 : (rs + 1) * 128],
                                wo_sb[k][:, ch * 512 : (ch + 1) * 512],
                                start=False,
                                stop=(k == KT - 1),
                            )
                    o_sb = ostage.tile([128, DIM], F32, tag="ost")
                    for ch in range(2):
                        nc.scalar.copy(
                            o_sb[:, ch * 512 : (ch + 1) * 512], o_ps[ch][:]
                        )
                    nc.gpsimd.dma_start(
                        d_out[rs * 128 : (rs + 1) * 128, :], o_sb[:]
                    )

    nc.compile()
    return nc


_NC_CACHE = {}


def _get_nc(x_bias_nonzero, c_bias_nonzero):
    key = (x_bias_nonzero, c_bias_nonzero)
    if key not in _NC_CACHE:
        _NC_CACHE[key] = build_kernel(*key)
    return _NC_CACHE[key]


def make_in_maps(x, context, norm_g, norm_b, cnorm_g, cnorm_b, Wq, Wkv, Wo, Wff1, Wff2):
    x = np.asarray(x, np.float32)
    context = np.asarray(context, np.float32)
    norm_g = np.asarray(norm_g, np.float32)
    norm_b = np.asarray(norm_b, np.float32)
    cnorm_g = np.asarray(cnorm_g, np.float32)
    cnorm_b = np.asarray(cnorm_b, np.float32)
    scale = DH ** -0.5
    bf = ml_dtypes.bfloat16
    wq = np.ascontiguousarray((norm_g[:, None] * np.asarray(Wq, np.float32)) * scale).astype(bf)
    wkv = np.ascontiguousarray(cnorm_g[:, None] * np.asarray(Wkv, np.float32)).astype(bf)
    wo = np.ascontiguousarray(np.asarray(Wo, np.float32)).astype(bf)
    wff1 = np.ascontiguousarray(norm_g[:, None] * np.asarray(Wff1, np.float32)).astype(bf)
    wff2 = np.ascontiguousarray(np.asarray(Wff2, np.float32)).astype(bf)
    x_bias = bool(np.any(norm_b != 0.0))
    c_bias = bool(np.any(cnorm_b != 0.0))
    in_maps = []
    for c in range(N_CORES):
        b = c // (N_CORES // B)
        r0 = (c % (N_CORES // B)) * R
        m = {
            "x": np.ascontiguousarray(x[b, r0 : r0 + R, :]),
            "ctx": np.ascontiguousarray(context[b]),
            "wq": wq,
            "wkv": wkv,
            "wo": wo,
            "wff1": wff1,
            "wff2": wff2,
        }
        if x_bias:
            m["xb"] = norm_b.reshape(1, DIM).copy()
        if c_bias:
            m["cb"] = cnorm_b.reshape(1, DIM).copy()
        in_maps.append(m)
    return in_maps, x_bias, c_bias


def gather_output(results):
    out = np.empty((B, N, DIM), np.float32)
    for c in range(N_CORES):
        b = c // (N_CORES // B)
        r0 = (c % (N_CORES // B)) * R
        out[b, r0 : r0 + R, :] = results[c]["out"]
    return out


def kernel(**inputs):
    from concourse.bass_utils import run_bass_kernel_spmd

    in_maps, x_bias, c_bias = make_in_maps(**inputs)
    nc = _get_nc(x_bias, c_bias)
    res = run_bass_kernel_spmd(nc, in_maps, list(range(N_CORES)))
    return gather_output(res.results)


# revision 16
# speedup vs baseline: 1.0688x; 1.0688x over previous
"""Trainium2 Bass kernel for nn_CrossAttention_65051574665735.

Cross-attention block (MQA, shared K/V head) + parallel SwiGLU FF.
Data-parallel over B*N rows across 8 NeuronCores: core c handles batch c//4,
rows (c%4)*512. Context + weights replicated (weights pre-cast to bf16 with the
layernorm scale g and the 1/sqrt(dh) attention scale folded in on the host).
No cross-core collectives; the host concatenates the 8 output slices.

Schedule: prologue (x-LN/transpose, Q proj, ctx-LN/KV) -> attention (ACT
exp-bound; score matmul pairs run row-tiled concurrently; FF1 matmuls
interleaved as PE filler) -> tail (FF2+Wo fused accumulation per row-slice).
"""

import sys

if "/opt/trn_rl_repo" not in sys.path:
    sys.path.insert(0, "/opt/trn_rl_repo")

import numpy as np
import ml_dtypes

import concourse.bass as bass
import concourse.tile as tile
from concourse import mybir, bacc
from concourse.masks import make_identity

F32 = mybir.dt.float32
BF16 = mybir.dt.bfloat16

B, N, J = 2, 2048, 2048
DIM, HEADS, DH = 1024, 16, 64
INNER = HEADS * DH
FF = 4 * DIM
EPS = 1e-5
N_CORES = 8
R = B * N // N_CORES  # 512 rows per core
KT = DIM // 128  # 8 contraction tiles over dim
RT = R // 128  # 4 row tiles
CT = J // 128  # 16 context row tiles
FT = FF // 128  # 32 ff tiles
AF = mybir.ActivationFunctionType


def _ln_tile(nc, pools, src_dram, t, dst_tiles, dst_col0, bias_tile):
    """LN one 128-row tile of src_dram; write transposed bf16 into dst_tiles.

    dst_tiles[k][:, dst_col0:dst_col0+128] gets chunk k of the transposed
    normalized rows. Stats on DVE, normalize on ACT, transposes on PE,
    psum->sbuf copies on DVE.
    """
    ln_pool, stats_pool, psum_tr, ident, eps_tile = pools
    x_t = ln_pool.tile([128, DIM], F32, tag="ln_x")
    nc.gpsimd.dma_start(x_t[:], src_dram[t * 128 : (t + 1) * 128, :])
    stats = stats_pool.tile([128, 2, nc.vector.BN_STATS_DIM], F32, tag="st")
    nc.vector.bn_stats(stats[:, 0, :], x_t[:, 0:512])
    nc.vector.bn_stats(stats[:, 1, :], x_t[:, 512:1024])
    mv = stats_pool.tile([128, nc.vector.BN_AGGR_DIM], F32, tag="mv")
    nc.vector.bn_aggr(mv[:], stats[:])
    rstd = stats_pool.tile([128, 1], F32, tag="rs")
    nc.scalar.activation(rstd[:], mv[:, 1:2], AF.Sqrt, bias=eps_tile[:])
    nc.vector.reciprocal_approx_fast(rstd[:], rstd[:])
    nmr = stats_pool.tile([128, 1], F32, tag="nmr")
    nc.vector.tensor_scalar(
        out=nmr[:],
        in0=mv[:, 0:1],
        scalar1=rstd[:, 0:1],
        scalar2=-1.0,
        op0=mybir.AluOpType.mult,
        op1=mybir.AluOpType.mult,
    )
    xn_t = ln_pool.tile([128, DIM], BF16, tag="ln_xn")
    nc.scalar.activation(
        xn_t[:], x_t[:], AF.Identity, bias=nmr[:, 0:1], scale=rstd[:, 0:1]
    )
    if bias_tile is not None:
        nc.vector.tensor_add(xn_t[:], xn_t[:], bias_tile[:])
    for k in range(KT):
        ps = psum_tr.tile([128, 128], BF16, tag="tr")
        nc.tensor.transpose(ps[:], xn_t[:, k * 128 : (k + 1) * 128], ident[:])
        nc.vector.tensor_copy(
            dst_tiles[k][:, dst_col0 : dst_col0 + 128], ps[:]
        )


def build_kernel(x_bias_nonzero: bool, c_bias_nonzero: bool):
    nc = bacc.Bacc(
        "TRN2", target_bir_lowering=False, debug=False, num_devices=N_CORES
    )
    d_x = nc.dram_tensor("x", [R, DIM], F32, kind="ExternalInput").ap()
    d_ctx = nc.dram_tensor("ctx", [J, DIM], F32, kind="ExternalInput").ap()
    d_wq = nc.dram_tensor("wq", [DIM, INNER], BF16, kind="ExternalInput").ap()
    d_wkv = nc.dram_tensor("wkv", [DIM, 2 * DH], BF16, kind="ExternalInput").ap()
    d_wo = nc.dram_tensor("wo", [INNER, DIM], BF16, kind="ExternalInput").ap()
    d_wff1 = nc.dram_tensor("wff1", [DIM, 2 * FF], BF16, kind="ExternalInput").ap()
    d_wff2 = nc.dram_tensor("wff2", [FF, DIM], BF16, kind="ExternalInput").ap()
    d_xb = (
        nc.dram_tensor("xb", [1, DIM], F32, kind="ExternalInput").ap()
        if x_bias_nonzero
        else None
    )
    d_cb = (
        nc.dram_tensor("cb", [1, DIM], F32, kind="ExternalInput").ap()
        if c_bias_nonzero
        else None
    )
    d_out = nc.dram_tensor("out", [R, DIM], F32, kind="ExternalOutput").ap()

    with tile.TileContext(nc) as tc:
        with (
            tc.tile_pool(name="consts", bufs=1) as consts,
            tc.tile_pool(name="persist", bufs=1) as persist,
            tc.tile_pool(name="ln", bufs=3) as ln_pool,
            tc.tile_pool(name="stats", bufs=4) as stats_pool,
        ):
            ident = consts.tile([128, 128], BF16)
            make_identity(nc, ident)
            eps_tile = consts.tile([128, 1], F32, tag="eps")
            nc.vector.memset(eps_tile[:], EPS)

            xb_tile = cb_tile = None
            if d_xb is not None:
                xb_tile = consts.tile([128, DIM], F32, tag="xb")
                nc.gpsimd.dma_start(
                    xb_tile[:],
                    bass.AP(
                        tensor=d_xb.tensor, offset=d_xb.offset,
                        ap=[[0, 128]] + d_xb.ap[1:],
                    ),
                )
            if d_cb is not None:
                cb_tile = consts.tile([128, DIM], F32, tag="cb")
                nc.gpsimd.dma_start(
                    cb_tile[:],
                    bass.AP(
                        tensor=d_cb.tensor, offset=d_cb.offset,
                        ap=[[0, 128]] + d_cb.ap[1:],
                    ),
                )

            xnT = [
                persist.tile([128, R], BF16, tag=f"xnT{k}", name=f"xnT{k}")
                for k in range(KT)
            ]
            kT = persist.tile([128, J], BF16, tag="kT")
            vo = [
                persist.tile([128, DH + 1], BF16, tag=f"vo{j}", name=f"vo{j}")
                for j in range(CT)
            ]
            aoT = [
                persist.tile([128, R], BF16, tag=f"aoT{k}", name=f"aoT{k}")
                for k in range(KT)
            ]
            hT = [
                persist.tile([128, R], BF16, tag=f"hT{f}", name=f"hT{f}")
                for f in range(FT)
            ]
            qT = [
                persist.tile([128, R], BF16, tag=f"qT{h}", name=f"qT{h}")
                for h in range(HEADS // 2)
            ]

            # ---- FF1 pools span prologue + attention (PE gap filler) ----
            with (
                tc.tile_pool(name="wff1", bufs=24) as wff1_pool,
                tc.tile_pool(name="sg", bufs=3) as sg_pool,
                tc.tile_pool(name="psF", bufs=1, space="PSUM") as psum_f,
            ):

                def ff1_block(w1a, w1g, fl, fi):
                    # h = a*g*sigmoid(g) = (a*g) / (1 + exp(-g)); the exp
                    # shares the ACT table with the attention exps (no
                    # ACT_TABLE_LOAD thrash, unlike Silu/Sigmoid)
                    a_ps = psum_f.tile([128, R], F32, tag="ffa", name="a_ps")
                    g_ps = psum_f.tile([128, R], F32, tag="ffg", name="g_ps")
                    for k in range(KT):
                        nc.tensor.matmul(
                            a_ps[:],
                            w1a[k][:, fl * 128 : (fl + 1) * 128],
                            xnT[k][:],
                            start=(k == 0),
                            stop=(k == KT - 1),
                        )
                    for k in range(KT):
                        nc.tensor.matmul(
                            g_ps[:],
                            w1g[k][:, fl * 128 : (fl + 1) * 128],
                            xnT[k][:],
                            start=(k == 0),
                            stop=(k == KT - 1),
                        )
                    gsb = sg_pool.tile([128, R], F32, tag="gsb", name="gsb")
                    nc.vector.tensor_copy(gsb[:], g_ps[:])
                    ag = sg_pool.tile([128, R], F32, tag="ag", name="ag")
                    nc.vector.tensor_mul(ag[:], a_ps[:], gsb[:])
                    eg = sg_pool.tile([128, R], F32, tag="eg", name="eg")
                    nc.scalar.activation(eg[:], gsb[:], AF.Exp, scale=-1.0)
                    nc.vector.tensor_scalar(
                        out=eg[:],
                        in0=eg[:],
                        scalar1=1.0,
                        scalar2=None,
                        op0=mybir.AluOpType.add,
                    )
                    nc.vector.reciprocal_approx_fast(eg[:], eg[:])
                    nc.vector.tensor_mul(hT[fi][:], ag[:], eg[:])

                def load_w1_group(g):
                    pair = ([], [])
                    for k in range(KT):
                        ta = wff1_pool.tile(
                            [128, 1024], BF16, tag="w1", name=f"w1a{g}_{k}"
                        )
                        nc.sync.dma_start(
                            ta[:],
                            d_wff1[
                                k * 128 : (k + 1) * 128,
                                g * 1024 : (g + 1) * 1024,
                            ],
                        )
                        pair[0].append(ta)
                        tg = wff1_pool.tile(
                            [128, 1024], BF16, tag="w1", name=f"w1g{g}_{k}"
                        )
                        nc.sync.dma_start(
                            tg[:],
                            d_wff1[
                                k * 128 : (k + 1) * 128,
                                FF + g * 1024 : FF + (g + 1) * 1024,
                            ],
                        )
                        pair[1].append(tg)
                    return pair

                w1 = {}
                with tc.tile_pool(name="psA", bufs=2, space="PSUM") as psum_tr:
                    pools = (ln_pool, stats_pool, psum_tr, ident, eps_tile)
                    # ---- Prologue: LN+transpose x and ctx, KV, Q ----
                    with (
                        tc.tile_pool(name="wq", bufs=1) as wq_pool,
                        tc.tile_pool(name="cnT", bufs=1) as cnT_pool,
                        tc.tile_pool(name="wkv", bufs=1) as wkv_pool,
                        tc.tile_pool(name="psKV", bufs=2, space="PSUM") as psum_kv,
                        tc.tile_pool(name="psQ", bufs=2, space="PSUM") as psum_q,
                        tc.tile_pool(name="vstage", bufs=2) as vstage,
                    ):
                        wq_sb = [
                            wq_pool.tile(
                                [128, INNER], BF16, tag=f"wq{k}", name=f"wq{k}"
                            )
                            for k in range(KT)
                        ]
                        for k in range(KT):
                            nc.sync.dma_start(
                                wq_sb[k][:], d_wq[k * 128 : (k + 1) * 128, :]
                            )
                        wkv_sb = [
                            wkv_pool.tile(
                                [128, 2 * DH], BF16, tag=f"wkv{k}", name=f"wkv{k}"
                            )
                            for k in range(KT)
                        ]
                        for k in range(KT):
                            nc.sync.dma_start(
                                wkv_sb[k][:], d_wkv[k * 128 : (k + 1) * 128, :]
                            )
                        # FF1 weight groups 0/1 stream during the prologue
                        w1[0] = load_w1_group(0)
                        w1[1] = load_w1_group(1)
                        for t in range(RT):
                            _ln_tile(nc, pools, d_x, t, xnT, t * 128, xb_tile)
                        cnT = [
                            cnT_pool.tile(
                                [128, J], BF16, tag=f"cnT{k}", name=f"cnT{k}"
                            )
                            for k in range(KT)
                        ]
                        for t in range(CT):
                            _ln_tile(nc, pools, d_ctx, t, cnT, t * 128, cb_tile)
                        for jc in range(CT):
                            nc.vector.memset(vo[jc][:, DH : DH + 1], 1.0)
                        for c in range(J // 512):
                            kv_ps = psum_kv.tile(
                                [128, 512], F32, tag="kv", name="kv_ps"
                            )
                            for k in range(KT):
                                nc.tensor.matmul(
                                    kv_ps[:],
                                    wkv_sb[k][:],
                                    cnT[k][:, c * 512 : (c + 1) * 512],
                                    start=(k == 0),
                                    stop=(k == KT - 1),
                                )
                            # rows 0:64 = kT chunk; duplicate into 64:128 for
                            # the row-tiled h2=1 score matmuls
                            nc.scalar.copy(
                                kT[0:DH, c * 512 : (c + 1) * 512], kv_ps[0:DH, :]
                            )
                            nc.gpsimd.dma_start(
                                kT[DH:128, c * 512 : (c + 1) * 512],
                                kT[0:DH, c * 512 : (c + 1) * 512],
                            )
                            vT_sb = vstage.tile([128, 512], BF16, tag="vT")
                            nc.vector.tensor_copy(
                                vT_sb[DH:128, :], kv_ps[DH:128, :]
                            )
                            for j4 in range(4):
                                jc = c * 4 + j4
                                vps = psum_tr.tile([128, DH], BF16, tag="tr")
                                nc.tensor.transpose(
                                    vps[:],
                                    vT_sb[DH:128, j4 * 128 : (j4 + 1) * 128],
                                    ident[DH:128, DH:128],
                                )
                                nc.vector.tensor_copy(vo[jc][:, 0:DH], vps[:])
                        # Q projections last in the prologue: PE filler while
                        # DVE/ACT finish the ctx LN chain
                        for hp in range(HEADS // 2):
                            q_ps = psum_q.tile(
                                [128, R], F32, tag="q", name="q_ps"
                            )
                            for k in range(KT):
                                nc.tensor.matmul(
                                    q_ps[:],
                                    wq_sb[k][:, hp * 128 : (hp + 1) * 128],
                                    xnT[k][:],
                                    start=(k == 0),
                                    stop=(k == KT - 1),
                                )
                            nc.vector.tensor_copy(qT[hp][:], q_ps[:])

                # ---- Attention (exp-bound) with FF1 blocks interleaved ----
                with (
                    tc.tile_pool(name="attnT", bufs=4) as attn_pool,
                    tc.tile_pool(name="smx", bufs=4) as smx_pool,
                    tc.tile_pool(name="psS", bufs=2, space="PSUM") as psum_s,
                    tc.tile_pool(name="psAV", bufs=2, space="PSUM") as psum_av,
                ):

                    def attn_block(hp):
                        a0s, a1s = [], []
                        for jp in range(CT // 2):
                            s0 = psum_s.tile(
                                [128, 2 * 512], F32, tag="sim", name="s0"
                            )
                            s1 = psum_s.tile(
                                [128, 2 * 512], F32, tag="sim", name="s1"
                            )
                            for half in range(2):
                                jc = 2 * jp + half
                                nc.tensor.matmul(
                                    s0[:, half * 512 : (half + 1) * 512],
                                    kT[0:DH, jc * 128 : (jc + 1) * 128],
                                    qT[hp][0:DH, :],
                                    start=True,
                                    stop=True,
                                )
                                nc.tensor.matmul(
                                    s1[:, half * 512 : (half + 1) * 512],
                                    kT[DH:128, jc * 128 : (jc + 1) * 128],
                                    qT[hp][DH:128, :],
                                    start=True,
                                    stop=True,
                                )
                            a0 = attn_pool.tile(
                                [128, 2 * 512], BF16, tag="attnT", name="a0"
                            )
                            nc.scalar.activation(a0[:], s0[:], AF.Exp)
                            a1 = attn_pool.tile(
                                [128, 2 * 512], BF16, tag="attnT", name="a1"
                            )
                            nc.scalar.activation(a1[:], s1[:], AF.Exp)
                            a0s.append(a0)
                            a1s.append(a1)
                        av0 = psum_av.tile([DH + 1, R], F32, tag="av", name="av0")
                        av1 = psum_av.tile([DH + 1, R], F32, tag="av", name="av1")
                        for jc in range(CT):
                            nc.tensor.matmul(
                                av0[:],
                                vo[jc][:],
                                a0s[jc // 2][
                                    :, (jc % 2) * 512 : (jc % 2 + 1) * 512
                                ],
                                start=(jc == 0),
                                stop=(jc == CT - 1),
                            )
                        for jc in range(CT):
                            nc.tensor.matmul(
                                av1[:],
                                vo[jc][:],
                                a1s[jc // 2][
                                    :, (jc % 2) * 512 : (jc % 2 + 1) * 512
                                ],
                                start=(jc == 0),
                                stop=(jc == CT - 1),
                            )
                        for h2, av in ((0, av0), (1, av1)):
                            srow = smx_pool.tile([1, R], F32, tag="srow")
                            nc.vector.tensor_copy(srow[:], av[DH : DH + 1, :])
                            rbc = smx_pool.tile([DH, R], F32, tag="rbc")
                            nc.gpsimd.partition_broadcast(rbc[:], srow[:])
                            nc.vector.reciprocal_approx_fast(rbc[:], rbc[:])
                            if h2 == 0:
                                nc.vector.tensor_mul(
                                    aoT[hp][0:DH, :], av[0:DH, :], rbc[:]
                                )
                            else:
                                tmp = smx_pool.tile([DH, R], BF16, tag="aotmp")
                                nc.vector.tensor_mul(tmp[:], av[0:DH, :], rbc[:])
                                nc.gpsimd.dma_start(aoT[hp][DH:128, :], tmp[:])

                    # FF1: 2 blocks per head-pair during attention (balances
                    # PE vs the ACT exp stream); the rest right after
                    ff_sched = [(g, fl) for g in range(4) for fl in range(8)]
                    fi_iter = iter(ff_sched)
                    for hp in range(HEADS // 2):
                        if hp == 2:
                            w1[2] = load_w1_group(2)
                        if hp == 4:
                            w1[3] = load_w1_group(3)
                        attn_block(hp)
                        for _ in range(2):
                            gfl = next(fi_iter, None)
                            if gfl is None:
                                continue
                            g, fl = gfl
                            ff1_block(w1[g][0], w1[g][1], fl, g * 8 + fl)
                    for gfl in fi_iter:
                        g, fl = gfl
                        ff1_block(w1[g][0], w1[g][1], fl, g * 8 + fl)

            # ---- Tail: FF2 (start) + Wo (stop) per row-slice ----
            with (
                tc.tile_pool(name="wff2", bufs=1) as wff2_pool,
                tc.tile_pool(name="wo", bufs=1) as wo_pool,
                tc.tile_pool(name="psO", bufs=4, space="PSUM") as psum_o,
                tc.tile_pool(name="ostage", bufs=2) as ostage,
            ):
                w2 = []
                for fi in range(FT):
                    t2 = wff2_pool.tile(
                        [128, DIM], BF16, tag=f"w2_{fi}", name=f"w2_{fi}"
                    )
                    nc.sync.dma_start(
                        t2[:], d_wff2[fi * 128 : (fi + 1) * 128, :]
                    )
                    w2.append(t2)
                wo_sb = [
                    wo_pool.tile([128, DIM], BF16, tag=f"wo{k}", name=f"wo{k}")
                    for k in range(KT)
                ]
                for k in range(KT):
                    nc.sync.dma_start(
                        wo_sb[k][:], d_wo[k * 128 : (k + 1) * 128, :]
                    )
                for rs in range(RT):
                    o_ps = [
                        psum_o.tile([128, 512], F32, tag="o", name=f"o{rs}{ch}")
                        for ch in range(2)
                    ]
                    for fi in range(FT):
                        for ch in range(2):
                            nc.tensor.matmul(
                                o_ps[ch][:],
                                hT[fi][:, rs * 128 : (rs + 1) * 128],
                                w2[fi][:, ch * 512 : (ch + 1) * 512],
                                start=(fi == 0),
                                stop=False,
                            )
                    for k in range(KT):
                        for ch in range(2):
                            nc.tensor.matmul(
                                o_ps[ch][:],
                                aoT[k][:, rs * 128 : (rs + 1) * 128],
                                wo_sb[k][:, ch * 512 : (ch + 1) * 512],
                                start=False,
                                stop=(k == KT - 1),
                            )
                    o_sb = ostage.tile([128, DIM], F32, tag="ost")
                    for ch in range(2):
                        nc.scalar.copy(
                            o_sb[:, ch * 512 : (ch + 1) * 512], o_ps[ch][:]
                        )
                    nc.gpsimd.dma_start(
                        d_out[rs * 128 : (rs + 1) * 128, :], o_sb[:]
                    )

    nc.compile()
    return nc


_NC_CACHE = {}


def _get_nc(x_bias_nonzero, c_bias_nonzero):
    key = (x_bias_nonzero, c_bias_nonzero)
    if key not in _NC_CACHE:
        _NC_CACHE[key] = build_kernel(*key)
    return _NC_CACHE[key]


def make_in_maps(x, context, norm_g, norm_b, cnorm_g, cnorm_b, Wq, Wkv, Wo, Wff1, Wff2):
    x = np.asarray(x, np.float32)
    context = np.asarray(context, np.float32)
    norm_g = np.asarray(norm_g, np.float32)
    norm_b = np.asarray(norm_b, np.float32)
    cnorm_g = np.asarray(cnorm_g, np.float32)
    cnorm_b = np.asarray(cnorm_b, np.float32)
    scale = DH ** -0.5
    bf = ml_dtypes.bfloat16
    wq = np.ascontiguousarray((norm_g[:, None] * np.asarray(Wq, np.float32)) * scale).astype(bf)
    wkv = np.ascontiguousarray(cnorm_g[:, None] * np.asarray(Wkv, np.float32)).astype(bf)
    wo = np.ascontiguousarray(np.asarray(Wo, np.float32)).astype(bf)
    wff1 = np.ascontiguousarray(norm_g[:, None] * np.asarray(Wff1, np.float32)).astype(bf)
    wff2 = np.ascontiguousarray(np.asarray(Wff2, np.float32)).astype(bf)
    x_bias = bool(np.any(norm_b != 0.0))
    c_bias = bool(np.any(cnorm_b != 0.0))
    in_maps = []
    for c in range(N_CORES):
        b = c // (N_CORES // B)
        r0 = (c % (N_CORES // B)) * R
        m = {
            "x": np.ascontiguousarray(x[b, r0 : r0 + R, :]),
            "ctx": np.ascontiguousarray(context[b]),
            "wq": wq,
            "wkv": wkv,
            "wo": wo,
            "wff1": wff1,
            "wff2": wff2,
        }
        if x_bias:
            m["xb"] = norm_b.reshape(1, DIM).copy()
        if c_bias:
            m["cb"] = cnorm_b.reshape(1, DIM).copy()
        in_maps.append(m)
    return in_maps, x_bias, c_bias


def gather_output(results):
    out = np.empty((B, N, DIM), np.float32)
    for c in range(N_CORES):
        b = c // (N_CORES // B)
        r0 = (c % (N_CORES // B)) * R
        out[b, r0 : r0 + R, :] = results[c]["out"]
    return out


def kernel(**inputs):
    from concourse.bass_utils import run_bass_kernel_spmd

    in_maps, x_bias, c_bias = make_in_maps(**inputs)
    nc = _get_nc(x_bias, c_bias)
    res = run_bass_kernel_spmd(nc, in_maps, list(range(N_CORES)))
    return gather_output(res.results)


# revision 18
# speedup vs baseline: 1.1533x; 1.0791x over previous
"""Trainium2 Bass kernel for nn_CrossAttention_65051574665735.

Cross-attention block (MQA, shared K/V head) + parallel SwiGLU FF.
Data-parallel over B*N rows across 8 NeuronCores: core c handles batch c//4,
rows (c%4)*512. Context + weights replicated (weights pre-cast to bf16 with the
layernorm scale g and the 1/sqrt(dh) attention scale folded in on the host).
No cross-core collectives; the host concatenates the 8 output slices.

Schedule: prologue (x/ctx LN+transpose, KV, Q; transposes batched 4-per-PSUM
tile) -> attention (ACT exp-bound; score matmul pairs row-tiled concurrently,
AV interleaved per jc; FF1 blocks as PE filler) -> FF1 leftovers -> tail
(FF2+Wo fused accumulation per row-slice). The only ACT table funcs used
after the prologue are Exp/Copy/Identity (one table - no reload thrash);
SwiGLU sigmoid is computed from Exp + DVE reciprocal_approx_fast.
"""

import sys

if "/opt/trn_rl_repo" not in sys.path:
    sys.path.insert(0, "/opt/trn_rl_repo")

import numpy as np
import ml_dtypes

import concourse.bass as bass
import concourse.tile as tile
from concourse import mybir, bacc
from concourse.masks import make_identity

F32 = mybir.dt.float32
BF16 = mybir.dt.bfloat16

B, N, J = 2, 2048, 2048
DIM, HEADS, DH = 1024, 16, 64
INNER = HEADS * DH
FF = 4 * DIM
EPS = 1e-5
N_CORES = 8
R = B * N // N_CORES  # 512 rows per core
KT = DIM // 128  # 8 contraction tiles over dim
RT = R // 128  # 4 row tiles
CT = J // 128  # 16 context row tiles
FT = FF // 128  # 32 ff tiles
AF = mybir.ActivationFunctionType


def _ln_tile(nc, pools, src_dram, t, dst, bias_tile, copy_on_act):
    """LN one 128-row tile of src_dram; write transposed bf16 into dst.

    dst is a [128, KT, cols] tile; chunk k of the transposed normalized rows
    lands at dst[:, k, t*128:(t+1)*128]. Stats on DVE, normalize on ACT
    (Identity: shares the table with Exp/Copy), transposes on PE batched
    4-per-PSUM-tile, PSUM->SBUF copies alternate ACT/DVE.
    """
    ln_pool, stats_pool, psum_tr, ident, eps_tile = pools
    x_t = ln_pool.tile([128, DIM], F32, tag="ln_x")
    nc.gpsimd.dma_start(x_t[:], src_dram[t * 128 : (t + 1) * 128, :])
    stats = stats_pool.tile([128, 2, nc.vector.BN_STATS_DIM], F32, tag="st")
    nc.vector.bn_stats(stats[:, 0, :], x_t[:, 0:512])
    nc.vector.bn_stats(stats[:, 1, :], x_t[:, 512:1024])
    mv = stats_pool.tile([128, nc.vector.BN_AGGR_DIM], F32, tag="mv")
    nc.vector.bn_aggr(mv[:], stats[:])
    rstd = stats_pool.tile([128, 1], F32, tag="rs")
    nc.scalar.activation(rstd[:], mv[:, 1:2], AF.Sqrt, bias=eps_tile[:])
    nc.vector.reciprocal_approx_fast(rstd[:], rstd[:])
    nmr = stats_pool.tile([128, 1], F32, tag="nmr")
    nc.vector.tensor_scalar(
        out=nmr[:],
        in0=mv[:, 0:1],
        scalar1=rstd[:, 0:1],
        scalar2=-1.0,
        op0=mybir.AluOpType.mult,
        op1=mybir.AluOpType.mult,
    )
    xn_t = ln_pool.tile([128, DIM], BF16, tag="ln_xn")
    nc.scalar.activation(
        xn_t[:], x_t[:], AF.Identity, bias=nmr[:, 0:1], scale=rstd[:, 0:1]
    )
    if bias_tile is not None:
        nc.vector.tensor_add(xn_t[:], xn_t[:], bias_tile[:])
    for k0 in (0, 4):
        ps = psum_tr.tile([128, 4, 128], BF16, tag="tr")
        for k4 in range(4):
            k = k0 + k4
            nc.tensor.transpose(
                ps[:, k4, :], xn_t[:, k * 128 : (k + 1) * 128], ident[:]
            )
        dst_ap = dst[:, k0 : k0 + 4, t * 128 : (t + 1) * 128]
        if copy_on_act:
            nc.scalar.copy(dst_ap, ps[:])
        else:
            nc.vector.tensor_copy(dst_ap, ps[:])


def build_kernel(x_bias_nonzero: bool, c_bias_nonzero: bool):
    nc = bacc.Bacc(
        "TRN2", target_bir_lowering=False, debug=False, num_devices=N_CORES
    )
    d_x = nc.dram_tensor("x", [R, DIM], F32, kind="ExternalInput").ap()
    d_ctx = nc.dram_tensor("ctx", [J, DIM], F32, kind="ExternalInput").ap()
    d_wq = nc.dram_tensor("wq", [DIM, INNER], BF16, kind="ExternalInput").ap()
    d_wkv = nc.dram_tensor("wkv", [DIM, 2 * DH], BF16, kind="ExternalInput").ap()
    d_wo = nc.dram_tensor("wo", [INNER, DIM], BF16, kind="ExternalInput").ap()
    d_wff1 = nc.dram_tensor("wff1", [DIM, 2 * FF], BF16, kind="ExternalInput").ap()
    d_wff2 = nc.dram_tensor("wff2", [FF, DIM], BF16, kind="ExternalInput").ap()
    d_xb = (
        nc.dram_tensor("xb", [1, DIM], F32, kind="ExternalInput").ap()
        if x_bias_nonzero
        else None
    )
    d_cb = (
        nc.dram_tensor("cb", [1, DIM], F32, kind="ExternalInput").ap()
        if c_bias_nonzero
        else None
    )
    d_out = nc.dram_tensor("out", [R, DIM], F32, kind="ExternalOutput").ap()

    with tile.TileContext(nc) as tc:
        with (
            tc.tile_pool(name="consts", bufs=1) as consts,
            tc.tile_pool(name="persist", bufs=1) as persist,
            tc.tile_pool(name="ln", bufs=3) as ln_pool,
            tc.tile_pool(name="stats", bufs=4) as stats_pool,
        ):
            ident = consts.tile([128, 128], BF16)
            make_identity(nc, ident)
            eps_tile = consts.tile([128, 1], F32, tag="eps")
            nc.vector.memset(eps_tile[:], EPS)

            xb_tile = cb_tile = None
            if d_xb is not None:
                xb_tile = consts.tile([128, DIM], F32, tag="xb")
                nc.gpsimd.dma_start(
                    xb_tile[:],
                    bass.AP(
                        tensor=d_xb.tensor, offset=d_xb.offset,
                        ap=[[0, 128]] + d_xb.ap[1:],
                    ),
                )
            if d_cb is not None:
                cb_tile = consts.tile([128, DIM], F32, tag="cb")
                nc.gpsimd.dma_start(
                    cb_tile[:],
                    bass.AP(
                        tensor=d_cb.tensor, offset=d_cb.offset,
                        ap=[[0, 128]] + d_cb.ap[1:],
                    ),
                )

            # xnT/cnT as single [128, KT, cols] tiles so a 4-chunk transpose
            # batch drains with one PSUM->SBUF copy
            xnT = persist.tile([128, KT, R], BF16, tag="xnT")
            kT = persist.tile([128, J], BF16, tag="kT")
            vo = [
                persist.tile([128, DH + 1], BF16, tag=f"vo{j}", name=f"vo{j}")
                for j in range(CT)
            ]
            aoT = [
                persist.tile([128, R], BF16, tag=f"aoT{k}", name=f"aoT{k}")
                for k in range(KT)
            ]
            hT = [
                persist.tile([128, R], BF16, tag=f"hT{f}", name=f"hT{f}")
                for f in range(FT)
            ]
            qT = [
                persist.tile([128, R], BF16, tag=f"qT{h}", name=f"qT{h}")
                for h in range(HEADS // 2)
            ]

            # ---- FF1 pools span prologue + attention (PE gap filler) ----
            with (
                tc.tile_pool(name="wff1", bufs=24) as wff1_pool,
                tc.tile_pool(name="sg", bufs=3) as sg_pool,
                tc.tile_pool(name="psF", bufs=1, space="PSUM") as psum_f,
            ):

                def ff1_block(pf, w1a, w1g, fl, fi):
                    # h = a*g*sigmoid(g) = (a*g) / (1 + exp(-g)); the exp
                    # shares the ACT table with the attention exps (no
                    # ACT_TABLE_LOAD thrash, unlike Silu/Sigmoid)
                    a_ps = pf.tile([128, R], F32, tag="ffa", name="a_ps")
                    g_ps = pf.tile([128, R], F32, tag="ffg", name="g_ps")
                    for k in range(KT):
                        nc.tensor.matmul(
                            a_ps[:],
                            w1a[k][:, fl * 128 : (fl + 1) * 128],
                            xnT[:, k, :],
                            start=(k == 0),
                            stop=(k == KT - 1),
                        )
                    for k in range(KT):
                        nc.tensor.matmul(
                            g_ps[:],
                            w1g[k][:, fl * 128 : (fl + 1) * 128],
                            xnT[:, k, :],
                            start=(k == 0),
                            stop=(k == KT - 1),
                        )
                    gsb = sg_pool.tile([128, R], F32, tag="gsb", name="gsb")
                    nc.vector.tensor_copy(gsb[:], g_ps[:])
                    ag = sg_pool.tile([128, R], F32, tag="ag", name="ag")
                    nc.vector.tensor_mul(ag[:], a_ps[:], gsb[:])
                    eg = sg_pool.tile([128, R], F32, tag="eg", name="eg")
                    nc.scalar.activation(eg[:], gsb[:], AF.Exp, scale=-1.0)
                    nc.vector.tensor_scalar(
                        out=eg[:],
                        in0=eg[:],
                        scalar1=1.0,
                        scalar2=None,
                        op0=mybir.AluOpType.add,
                    )
                    nc.vector.reciprocal_approx_fast(eg[:], eg[:])
                    nc.vector.tensor_mul(hT[fi][:], ag[:], eg[:])

                def load_w1_group(g):
                    pair = ([], [])
                    for k in range(KT):
                        ta = wff1_pool.tile(
                            [128, 1024], BF16, tag="w1", name=f"w1a{g}_{k}"
                        )
                        nc.sync.dma_start(
                            ta[:],
                            d_wff1[
                                k * 128 : (k + 1) * 128,
                                g * 1024 : (g + 1) * 1024,
                            ],
                        )
                        pair[0].append(ta)
                        tg = wff1_pool.tile(
                            [128, 1024], BF16, tag="w1", name=f"w1g{g}_{k}"
                        )
                        nc.sync.dma_start(
                            tg[:],
                            d_wff1[
                                k * 128 : (k + 1) * 128,
                                FF + g * 1024 : FF + (g + 1) * 1024,
                            ],
                        )
                        pair[1].append(tg)
                    return pair

                w1 = {}
                ff_sched = [(g, fl) for g in range(4) for fl in range(8)]
                fi_iter = iter(ff_sched)

                with tc.tile_pool(name="psA", bufs=2, space="PSUM") as psum_tr:
                    pools = (ln_pool, stats_pool, psum_tr, ident, eps_tile)
                    # ---- Prologue: LN+transpose x and ctx, KV, Q ----
                    with (
                        tc.tile_pool(name="wq", bufs=1) as wq_pool,
                        tc.tile_pool(name="cnT", bufs=1) as cnT_pool,
                        tc.tile_pool(name="wkv", bufs=1) as wkv_pool,
                        tc.tile_pool(name="psKV", bufs=2, space="PSUM") as psum_kv,
                        tc.tile_pool(name="psQ", bufs=1, space="PSUM") as psum_q,
                        tc.tile_pool(name="vstage", bufs=2) as vstage,
                    ):
                        wq_sb = [
                            wq_pool.tile(
                                [128, INNER], BF16, tag=f"wq{k}", name=f"wq{k}"
                            )
                            for k in range(KT)
                        ]
                        for k in range(KT):
                            nc.sync.dma_start(
                                wq_sb[k][:], d_wq[k * 128 : (k + 1) * 128, :]
                            )
                        wkv_sb = [
                            wkv_pool.tile(
                                [128, 2 * DH], BF16, tag=f"wkv{k}", name=f"wkv{k}"
                            )
                            for k in range(KT)
                        ]
                        for k in range(KT):
                            nc.sync.dma_start(
                                wkv_sb[k][:], d_wkv[k * 128 : (k + 1) * 128, :]
                            )
                        # FF1 weight groups 0/1 stream during the prologue
                        w1[0] = load_w1_group(0)
                        w1[1] = load_w1_group(1)
                        for t in range(RT):
                            _ln_tile(nc, pools, d_x, t, xnT, xb_tile, t % 2)
                        cnT = cnT_pool.tile([128, KT, J], BF16, tag="cnT")
                        for t in range(CT):
                            _ln_tile(nc, pools, d_ctx, t, cnT, cb_tile, t % 2)
                        for jc in range(CT):
                            nc.vector.memset(vo[jc][:, DH : DH + 1], 1.0)
                        for c in range(J // 512):
                            kv_ps = psum_kv.tile(
                                [128, 512], F32, tag="kv", name="kv_ps"
                            )
                            for k in range(KT):
                                nc.tensor.matmul(
                                    kv_ps[:],
                                    wkv_sb[k][:],
                                    cnT[:, k, c * 512 : (c + 1) * 512],
                                    start=(k == 0),
                                    stop=(k == KT - 1),
                                )
                            # rows 0:64 = kT chunk; duplicate into 64:128 for
                            # the row-tiled h2=1 score matmuls
                            nc.scalar.copy(
                                kT[0:DH, c * 512 : (c + 1) * 512], kv_ps[0:DH, :]
                            )
                            nc.gpsimd.dma_start(
                                kT[DH:128, c * 512 : (c + 1) * 512],
                                kT[0:DH, c * 512 : (c + 1) * 512],
                            )
                            vT_sb = vstage.tile([128, 512], BF16, tag="vT")
                            nc.vector.tensor_copy(
                                vT_sb[DH:128, :], kv_ps[DH:128, :]
                            )
                            for j4 in range(4):
                                jc = c * 4 + j4
                                vps = psum_tr.tile([128, DH], BF16, tag="tr")
                                nc.tensor.transpose(
                                    vps[:],
                                    vT_sb[DH:128, j4 * 128 : (j4 + 1) * 128],
                                    ident[DH:128, DH:128],
                                )
                                nc.vector.tensor_copy(vo[jc][:, 0:DH], vps[:])
                        # Q projections late in the prologue: PE filler while
                        # DVE/ACT finish the ctx LN chain
                        for hp in range(HEADS // 2):
                            q_ps = psum_q.tile(
                                [128, R], F32, tag="q", name="q_ps"
                            )
                            for k in range(KT):
                                nc.tensor.matmul(
                                    q_ps[:],
                                    wq_sb[k][:, hp * 128 : (hp + 1) * 128],
                                    xnT[:, k, :],
                                    start=(k == 0),
                                    stop=(k == KT - 1),
                                )
                            nc.vector.tensor_copy(qT[hp][:], q_ps[:])
                        # a few FF1 blocks to keep PE busy at the prologue
                        # tail (emitted after the last Sqrt: one table switch)
                        for _ in range(3):
                            g, fl = next(fi_iter)
                            ff1_block(psum_f, w1[g][0], w1[g][1], fl, g * 8 + fl)

                # ---- Attention (exp-bound) with FF1 blocks interleaved ----
                with (
                    tc.tile_pool(name="attnT", bufs=4) as attn_pool,
                    tc.tile_pool(name="smx", bufs=4) as smx_pool,
                    tc.tile_pool(name="psS", bufs=2, space="PSUM") as psum_s,
                    tc.tile_pool(name="psAV", bufs=2, space="PSUM") as psum_av,
                ):

                    def attn_block(hp):
                        av0 = psum_av.tile([DH + 1, R], F32, tag="av", name="av0")
                        av1 = psum_av.tile([DH + 1, R], F32, tag="av", name="av1")
                        for jp in range(CT // 2):
                            s0 = psum_s.tile(
                                [128, 2 * 512], F32, tag="sim", name="s0"
                            )
                            s1 = psum_s.tile(
                                [128, 2 * 512], F32, tag="sim", name="s1"
                            )
                            for half in range(2):
                                jc = 2 * jp + half
                                nc.tensor.matmul(
                                    s0[:, half * 512 : (half + 1) * 512],
                                    kT[0:DH, jc * 128 : (jc + 1) * 128],
                                    qT[hp][0:DH, :],
                                    start=True,
                                    stop=True,
                                )
                                nc.tensor.matmul(
                                    s1[:, half * 512 : (half + 1) * 512],
                                    kT[DH:128, jc * 128 : (jc + 1) * 128],
                                    qT[hp][DH:128, :],
                                    start=True,
                                    stop=True,
                                )
                            a0 = attn_pool.tile(
                                [128, 2 * 512], BF16, tag="attnT", name="a0"
                            )
                            nc.scalar.activation(a0[:], s0[:], AF.Exp)
                            a1 = attn_pool.tile(
                                [128, 2 * 512], BF16, tag="attnT", name="a1"
                            )
                            nc.scalar.activation(a1[:], s1[:], AF.Exp)
                            for half in range(2):
                                jc = 2 * jp + half
                                nc.tensor.matmul(
                                    av0[:],
                                    vo[jc][:],
                                    a0[:, half * 512 : (half + 1) * 512],
                                    start=(jc == 0),
                                    stop=(jc == CT - 1),
                                )
                                nc.tensor.matmul(
                                    av1[:],
                                    vo[jc][:],
                                    a1[:, half * 512 : (half + 1) * 512],
                                    start=(jc == 0),
                                    stop=(jc == CT - 1),
                                )
                        for h2, av in ((0, av0), (1, av1)):
                            srow = smx_pool.tile([1, R], F32, tag="srow")
                            nc.vector.tensor_copy(srow[:], av[DH : DH + 1, :])
                            rbc = smx_pool.tile([DH, R], F32, tag="rbc")
                            nc.gpsimd.partition_broadcast(rbc[:], srow[:])
                            nc.vector.reciprocal_approx_fast(rbc[:], rbc[:])
                            if h2 == 0:
                                nc.vector.tensor_mul(
                                    aoT[hp][0:DH, :], av[0:DH, :], rbc[:]
                                )
                            else:
                                tmp = smx_pool.tile([DH, R], BF16, tag="aotmp")
                                nc.vector.tensor_mul(tmp[:], av[0:DH, :], rbc[:])
                                nc.gpsimd.dma_start(aoT[hp][DH:128, :], tmp[:])

                    # FF1: 2 blocks per head-pair during attention (balances
                    # PE vs the ACT exp stream); the rest right after
                    for hp in range(HEADS // 2):
                        if hp == 2:
                            w1[2] = load_w1_group(2)
                        if hp == 4:
                            w1[3] = load_w1_group(3)
                        attn_block(hp)
                        for _ in range(2):
                            gfl = next(fi_iter, None)
                            if gfl is None:
                                continue
                            g, fl = gfl
                            ff1_block(psum_f, w1[g][0], w1[g][1], fl, g * 8 + fl)

                # FF1 leftovers after the attention PSUM pools close: a
                # deeper psF pool here keeps consecutive blocks pipelined
                with tc.tile_pool(name="psF2", bufs=2, space="PSUM") as psum_f2:
                    for gfl in fi_iter:
                        g, fl = gfl
                        ff1_block(psum_f2, w1[g][0], w1[g][1], fl, g * 8 + fl)

            # ---- Tail: FF2 (start) + Wo (stop) per row-slice ----
            with (
                tc.tile_pool(name="wff2", bufs=1) as wff2_pool,
                tc.tile_pool(name="wo", bufs=1) as wo_pool,
                tc.tile_pool(name="psO", bufs=4, space="PSUM") as psum_o,
                tc.tile_pool(name="ostage", bufs=2) as ostage,
            ):
                w2 = []
                for fi in range(FT):
                    t2 = wff2_pool.tile(
                        [128, DIM], BF16, tag=f"w2_{fi}", name=f"w2_{fi}"
                    )
                    nc.sync.dma_start(
                        t2[:], d_wff2[fi * 128 : (fi + 1) * 128, :]
                    )
                    w2.append(t2)
                wo_sb = [
                    wo_pool.tile([128, DIM], BF16, tag=f"wo{k}", name=f"wo{k}")
                    for k in range(KT)
                ]
                for k in range(KT):
                    nc.sync.dma_start(
                        wo_sb[k][:], d_wo[k * 128 : (k + 1) * 128, :]
                    )
                for rs in range(RT):
                    o_ps = [
                        psum_o.tile([128, 512], F32, tag="o", name=f"o{rs}{ch}")
                        for ch in range(2)
                    ]
                    for fi in range(FT):
                        for ch in range(2):
                            nc.tensor.matmul(
                                o_ps[ch][:],
                                hT[fi][:, rs * 128 : (rs + 1) * 128],
                                w2[fi][:, ch * 512 : (ch + 1) * 512],
                                start=(fi == 0),
                                stop=False,
                            )
                    for k in range(KT):
                        for ch in range(2):
                            nc.tensor.matmul(
                                o_ps[ch][:],
                                aoT[k][:, rs * 128 : (rs + 1) * 128],
                                wo_sb[k][:, ch * 512 : (ch + 1) * 512],
                                start=False,
                                stop=(k == KT - 1),
                            )
                    o_sb = ostage.tile([128, DIM], F32, tag="ost")
                    for ch in range(2):
                        nc.scalar.copy(
                            o_sb[:, ch * 512 : (ch + 1) * 512], o_ps[ch][:]
                        )
                    nc.gpsimd.dma_start(
                        d_out[rs * 128 : (rs + 1) * 128, :], o_sb[:]
                    )

    nc.compile()
    return nc


_NC_CACHE = {}


def _get_nc(x_bias_nonzero, c_bias_nonzero):
    key = (x_bias_nonzero, c_bias_nonzero)
    if key not in _NC_CACHE:
        _NC_CACHE[key] = build_kernel(*key)
    return _NC_CACHE[key]


def make_in_maps(x, context, norm_g, norm_b, cnorm_g, cnorm_b, Wq, Wkv, Wo, Wff1, Wff2):
    x = np.asarray(x, np.float32)
    context = np.asarray(context, np.float32)
    norm_g = np.asarray(norm_g, np.float32)
    norm_b = np.asarray(norm_b, np.float32)
    cnorm_g = np.asarray(cnorm_g, np.float32)
    cnorm_b = np.asarray(cnorm_b, np.float32)
    scale = DH ** -0.5
    bf = ml_dtypes.bfloat16
    wq = np.ascontiguousarray((norm_g[:, None] * np.asarray(Wq, np.float32)) * scale).astype(bf)
    wkv = np.ascontiguousarray(cnorm_g[:, None] * np.asarray(Wkv, np.float32)).astype(bf)
    wo = np.ascontiguousarray(np.asarray(Wo, np.float32)).astype(bf)
    wff1 = np.ascontiguousarray(norm_g[:, None] * np.asarray(Wff1, np.float32)).astype(bf)
    wff2 = np.ascontiguousarray(np.asarray(Wff2, np.float32)).astype(bf)
    x_bias = bool(np.any(norm_b != 0.0))
    c_bias = bool(np.any(cnorm_b != 0.0))
    in_maps = []
    for c in range(N_CORES):
        b = c // (N_CORES // B)
        r0 = (c % (N_CORES // B)) * R
        m = {
            "x": np.ascontiguousarray(x[b, r0 : r0 + R, :]),
            "ctx": np.ascontiguousarray(context[b]),
            "wq": wq,
            "wkv": wkv,
            "wo": wo,
            "wff1": wff1,
            "wff2": wff2,
        }
        if x_bias:
            m["xb"] = norm_b.reshape(1, DIM).copy()
        if c_bias:
            m["cb"] = cnorm_b.reshape(1, DIM).copy()
        in_maps.append(m)
    return in_maps, x_bias, c_bias


def gather_output(results):
    out = np.empty((B, N, DIM), np.float32)
    for c in range(N_CORES):
        b = c // (N_CORES // B)
        r0 = (c % (N_CORES // B)) * R
        out[b, r0 : r0 + R, :] = results[c]["out"]
    return out


def kernel(**inputs):
    from concourse.bass_utils import run_bass_kernel_spmd

    in_maps, x_bias, c_bias = make_in_maps(**inputs)
    nc = _get_nc(x_bias, c_bias)
    res = run_bass_kernel_spmd(nc, in_maps, list(range(N_CORES)))
    return gather_output(res.results)
